# revision 24
# baseline (speedup 1.0000x reference)
"""FPN RoIAlign pooler (maskrcnn-benchmark semantics) on 8 Trainium2 cores.

Strategy: data-parallel over RoIs. The host computes (from boxes/batch_ids
metadata only) each RoI's FPN level, feature-window origin/extent and a dense
interpolation matrix W[pix, 49] that folds bilinear weights + 2x2 sample
averaging. RoIs are packed into groups of 8 (one per core) with similar
window shapes, so one SPMD program (static shapes per slot = max over the
group) serves all cores; per-core variation flows through data tables
(window offsets, W values).

Device per slot: dynamic-offset DMA of the [2x128, KH, KW] feature window
into SBUF (channels on partitions), cast to bf16, TensorE transpose of
128-pixel chunks to pixel-major, then matmul cropT.T @ W accumulated into
PSUM [128, 49] per channel-block, copied into an output accumulator and
DMA'd out in groups.
"""

import random

import numpy as np

P = 7
SR = 2
S = P * SR
SCALES = (0.25, 0.125, 0.0625, 0.03125)
LSIZE = (200, 100, 50, 25)
C = 256
B = 2
NCORES = 8
NBINS = P * P
GRP = 8  # slots per W-prefetch / output-DMA group


def _map_levels(boxes):
    area = (boxes[:, 2] - boxes[:, 0]) * (boxes[:, 3] - boxes[:, 1])
    s = np.sqrt(np.maximum(area, 0.0))
    lvl = np.floor(4.0 + np.log2(s / 224.0 + 1e-6))
    return (np.clip(lvl, 2, 5) - 2).astype(np.int32)


def _plan(boxes, batch_ids):
    N = boxes.shape[0]
    boxes = boxes.astype(np.float64)
    lvl = _map_levels(boxes)
    bidx = batch_ids.astype(np.int64)

    sc = np.asarray(SCALES)[lvl]
    Ls = np.asarray(LSIZE)[lvl].astype(np.int64)  # H == W per level
    x1 = boxes[:, 0] * sc
    y1 = boxes[:, 1] * sc
    x2 = boxes[:, 2] * sc
    y2 = boxes[:, 3] * sc
    rw = np.maximum(x2 - x1, 1.0)
    rh = np.maximum(y2 - y1, 1.0)
    grid = (np.arange(P)[:, None] + (np.arange(SR)[None, :] + 0.5) / SR).reshape(-1)

    xs = x1[:, None] + grid[None, :] * (rw / P)[:, None]  # [N, S]
    ys = y1[:, None] + grid[None, :] * (rh / P)[:, None]
    Lf = Ls[:, None].astype(np.float64)
    vx = (xs > -1.0) & (xs < Lf)
    vy = (ys > -1.0) & (ys < Lf)
    xc = np.clip(xs, 0.0, Lf - 1.0)
    yc = np.clip(ys, 0.0, Lf - 1.0)
    xlo = np.minimum(np.floor(xc), Lf - 2.0).astype(np.int64)
    ylo = np.minimum(np.floor(yc), Lf - 2.0).astype(np.int64)
    fx = xc - xlo
    fy = yc - ylo

    # Tight window extents over valid samples (neighbors lo, lo+1).
    big = np.int64(1 << 40)
    x0 = np.where(vx, xlo, big).min(axis=1)
    xm = np.where(vx, xlo, -big).max(axis=1) + 1
    y0 = np.where(vy, ylo, big).min(axis=1)
    ym = np.where(vy, ylo, -big).max(axis=1) + 1
    dead = ~(vx.any(axis=1) & vy.any(axis=1))
    x0 = np.where(dead, 0, x0)
    xm = np.where(dead, 1, xm)
    y0 = np.where(dead, 0, y0)
    ym = np.where(dead, 1, ym)
    KW = (xm - x0 + 1).astype(np.int64)
    KH = (ym - y0 + 1).astype(np.int64)

    # --- pack RoIs into groups of NCORES with similar window shapes ---
    # DMA cost of a slot ~ KHmax*(KWmax + 24): one descriptor per (c, row),
    # each costing ~3 cycles + KW*4/32 beats.
    def gcost(ent):
        kh = max(KH[i] for i in ent)
        kw = max(KW[i] for i in ent)
        return kh * (kw + 24)

    groups = []
    for l in range(4):
        rem = [i for i in range(N) if lvl[i] == l]
        if not rem:
            continue
        rem.sort(key=lambda i: -(KH[i] * KW[i]))
        while rem:
            ent = [rem.pop(0)]
            while len(ent) < NCORES and rem:
                best = min(rem, key=lambda i: gcost(ent + [i]))
                rem.remove(best)
                ent.append(best)
            groups.append(ent)

    rng = random.Random(0)
    cost_cache = [gcost(e) for e in groups]
    ng = len(groups)
    for _ in range(200000):
        a = rng.randrange(ng)
        b = rng.randrange(ng)
        if a == b or lvl[groups[a][0]] != lvl[groups[b][0]]:
            continue
        ia = rng.randrange(len(groups[a]))
        ib = rng.randrange(len(groups[b]))
        ga = groups[a][:]
        gb = groups[b][:]
        ga[ia], gb[ib] = groups[b][ib], groups[a][ia]
        nc_ = gcost(ga) + gcost(gb)
        if nc_ < cost_cache[a] + cost_cache[b]:
            groups[a], groups[b] = ga, gb
            cost_cache[a], cost_cache[b] = gcost(ga), gcost(gb)

    # order: within each level, big slots first (stable pipeline warmup)
    groups.sort(key=lambda e: (lvl[e[0]], -gcost(e)))
    n_slots = len(groups)

    slot_lv = [int(lvl[e[0]]) for e in groups]
    slot_kh = [int(max(KH[i] for i in e)) for e in groups]
    slot_kw = [int(max(KW[i] for i in e)) for e in groups]

    pix = np.array([kh * kw for kh, kw in zip(slot_kh, slot_kw)], dtype=np.int64)
    nchunk = -(-pix // 128)
    wrows = nchunk * 128
    woff = np.concatenate([[0], np.cumsum(wrows)])

    tabs = np.zeros((NCORES, 1, n_slots), dtype=np.int32)
    totci = int(nchunk.sum())
    cioffs = np.concatenate([[0], np.cumsum(nchunk)]).astype(np.int64)
    wtabs = np.zeros((NCORES, 128, totci * NBINS), dtype=np.float32)
    assign = np.full((NCORES, n_slots), -1, dtype=np.int64)

    binq = (np.arange(S) // SR)[:, None] * P + (np.arange(S) // SR)[None, :]
    for t, ent in enumerate(groups):
        kh_s, kw_s = slot_kh[t], slot_kw[t]
        L = LSIZE[slot_lv[t]]
        HW = L * L
        for k in range(NCORES):
            if k >= len(ent):
                continue  # dummy: zero W, offset 0
            r = ent[k]
            assign[k, t] = r
            yo = min(int(y0[r]), L - kh_s)
            xo = min(int(x0[r]), L - kw_s)
            tabs[k, 0, t] = int(bidx[r]) * C * HW + yo * L + xo
            w = np.zeros((kh_s * kw_s, NBINS), dtype=np.float64)
            ry = (ylo[r] - yo).astype(np.int64)
            rx = (xlo[r] - xo).astype(np.int64)
            wy = np.stack([1.0 - fy[r], fy[r]])
            wx = np.stack([1.0 - fx[r], fx[r]])
            val = (vy[r][:, None] & vx[r][None, :]).astype(np.float64)
            for dy in range(2):
                for dx in range(2):
                    pixi = (ry + dy)[:, None] * kw_s + (rx + dx)[None, :]
                    wgt = wy[dy][:, None] * wx[dx][None, :] * 0.25 * val
                    np.add.at(w, (pixi.ravel(), binq.ravel()), wgt.ravel())
            wf = w.astype(np.float32)
            for ci in range(int(nchunk[t])):
                kk = min(128, kh_s * kw_s - ci * 128)
                col = (int(cioffs[t]) + ci) * NBINS
                wtabs[k, :kk, col : col + NBINS] = wf[ci * 128 : ci * 128 + kk]

    slots = [
        dict(
            lv=slot_lv[t],
            kh=slot_kh[t],
            kw=slot_kw[t],
            cio=int(cioffs[t]),
            nchunk=int(nchunk[t]),
        )
        for t in range(n_slots)
    ]
    return slots, tabs, wtabs, assign


# ---------------------------------------------------------------------------
# Device program
# ---------------------------------------------------------------------------

_MAXW = 1  # this walrus build allows 1 sem wait per TPB_CTRL (Drain)


def _patch_tile_drain():
    import concourse.tile as tile
    from bass_rust import ScopedClock

    if getattr(tile.TileContext, "_drain_patched", False):
        return

    def _drain_and_barrier(self, tick_clock, wait_clock):
        nc = self.nc
        drain_inst = nc.sync.drain()
        wait_clock.add_sem_waits(
            drain_inst.ins, ScopedClock({None: tick_clock.global_clock})
        )
        si = drain_inst.ins.sync_info
        ow = list(si.on_wait) if si is not None and si.on_wait else []
        if len(ow) > _MAXW:
            si.on_wait = ow[:_MAXW]
            for i in range(_MAXW, len(ow), _MAXW):
                d2 = nc.sync.drain()
                si2 = d2.ins.sync_info
                chunk = ow[i : i + _MAXW]
                if si2 is None:
                    d2.ins.sync_info = type(si)(on_wait=chunk, on_update=[])
                else:
                    si2.on_wait = chunk
        nc.all_engine_barrier()
        assert self.sems is not None
        popped = nc._tile_sem_poison_stack.pop()
        assert popped is self._sem_poison
        nc.clear_and_free_semaphores(list(self.sems.allocated().values()))
        nc.all_engine_barrier()

    tile.TileContext._drain_and_barrier = _drain_and_barrier
    tile.TileContext._drain_patched = True


def _load_idx(eng, ap, lo, hi, name):
    tmp = eng.alloc_register(f"idx_{name}")
    eng.reg_load(tmp, ap)
    return eng.snap(tmp, donate=True, min_val=lo, max_val=hi)


def _build_program(slots, wtab_cols, use_bf16=True, xpose_bf16=False, swdge=True, xbar=True):
    import concourse.bass as bass
    import concourse.tile as tile
    from concourse import bacc, mybir
    from concourse.masks import make_identity

    _patch_tile_drain()

    n_slots = len(slots)
    f32 = mybir.dt.float32
    cdt = mybir.dt.bfloat16 if use_bf16 else f32
    nc = bacc.Bacc("TRN2", target_bir_lowering=False, debug=False, num_devices=NCORES)

    feats = [
        nc.dram_tensor(f"feat{l}", [B, C, LSIZE[l], LSIZE[l]], f32, kind="ExternalInput")
        for l in range(4)
    ]
    wtab = nc.dram_tensor("wtab", [128, wtab_cols], cdt, kind="ExternalInput")
    tab = nc.dram_tensor("tab", [1, n_slots], mybir.dt.int32, kind="ExternalInput")
    out = nc.dram_tensor("out", [128, n_slots * 2 * NBINS], f32, kind="ExternalOutput")

    max_pix = max(s["kh"] * s["kw"] for s in slots)
    max_c16 = max(s["kh"] * s["kw"] + s["nchunk"] * 128 for s in slots)
    g_bounds = [(g0, min(g0 + GRP, n_slots)) for g0 in range(0, n_slots, GRP)]
    max_gci = max(sum(slots[t]["nchunk"] for t in range(g0, g1)) for g0, g1 in g_bounds)

    with tile.TileContext(nc) as tc:
        with (
            tc.tile_pool(name="const", bufs=1) as const_pool,
            tc.tile_pool(name="crop", bufs=6) as crop_pool,
            tc.tile_pool(name="crop16", bufs=4) as crop16_pool,
            tc.tile_pool(name="wsb", bufs=3) as w_pool,
            tc.tile_pool(name="ctp", bufs=12) as ct_pool,
            tc.tile_pool(name="oacc", bufs=3) as oacc_pool,
            tc.tile_pool(name="pt", bufs=4, space="PSUM") as pt_pool,
            tc.tile_pool(name="pout", bufs=4, space="PSUM") as pout_pool,
        ):
            xdt = cdt if xpose_bf16 else f32
            ident = const_pool.tile([128, 128], xdt)
            make_identity(nc, ident[:])
            tab_sb = const_pool.tile([1, n_slots], mybir.dt.int32)
            nc.sync.dma_start(tab_sb[:], tab.ap())

            oacc = None
            w_g = None
            cio = 0
            for t, sl in enumerate(slots):
                lv, kh, kw, nch = sl["lv"], sl["kh"], sl["kw"], sl["nchunk"]
                pix = kh * kw
                L = LSIZE[lv]
                HW = L * L
                gi = t // GRP
                if t % GRP == 0:
                    g0, g1 = g_bounds[gi]
                    gsz = g1 - g0
                    oacc = oacc_pool.tile([128, gsz * 2 * NBINS], f32, tag="oacc")
                    gci = sum(slots[u]["nchunk"] for u in range(g0, g1))
                    w_g = w_pool.tile([128, max_gci * NBINS], cdt, tag="wsb")
                    col0 = slots[g0]["cio"] * NBINS
                    nc.scalar.dma_start(
                        out=w_g[:, : gci * NBINS],
                        in_=wtab.ap()[:, col0 : col0 + gci * NBINS],
                    )
                    cio = 0

                max_off = (B - 1) * C * HW + (L - kh) * L + (L - kw)
                voff = _load_idx(nc.sync, tab_sb[0:1, t : t + 1], 0, max_off, f"o{t}")
                both_gp = swdge and t % 3 == 2
                if swdge:
                    voff2 = _load_idx(
                        nc.gpsimd, tab_sb[0:1, t : t + 1], 0, max_off, f"p{t}"
                    )
                else:
                    voff2 = voff
                crop = crop_pool.tile([128, 2 * max_pix], f32, tag="crop")
                for cb in range(2):
                    use_gp = swdge and (cb == 1 or both_gp)
                    src = bass.AP(
                        feats[lv],
                        (voff2 if use_gp else voff) + cb * 128 * HW,
                        [[HW, 128], [L, kh], [1, kw]],
                    )
                    eng = nc.gpsimd if use_gp else nc.sync
                    eng.dma_start(out=crop[:, cb * pix : (cb + 1) * pix], in_=src)

                if use_bf16 and (xpose_bf16 or xbar):
                    crop16 = crop16_pool.tile([128, max_c16], cdt, tag="crop16")
                    nc.vector.tensor_copy(
                        out=crop16[:, : 2 * pix], in_=crop[:, : 2 * pix]
                    )
                    tail = nch * 128 - pix
                    if xbar and tail > 0:
                        nc.vector.memset(crop16[:, 2 * pix : 2 * pix + tail], 0.0)
                    csrc = crop16
                else:
                    csrc = crop

                pouts = []
                for cb in range(2):
                    pout_cb = pout_pool.tile([128, NBINS], f32, tag="pout")
                    pouts.append(pout_cb)
                if xbar:
                    cropTs = []
                    for ci in range(nch):
                        for cb in range(2):
                            cropT = ct_pool.tile([128, 128], cdt, tag="ctp")
                            nc.scalar.dma_start(
                                out=cropT[:, :],
                                in_=csrc[
                                    :, cb * pix + ci * 128 : cb * pix + ci * 128 + 128
                                ],
                                transpose=True,
                            )
                            cropTs.append(cropT)
                    for ci in range(nch):
                        k = min(128, pix - ci * 128)
                        for cb in range(2):
                            nc.tensor.matmul(
                                out=pouts[cb][:],
                                lhsT=cropTs[ci * 2 + cb][:k, :],
                                rhs=w_g[
                                    :k, (cio + ci) * NBINS : (cio + ci + 1) * NBINS
                                ],
                                start=(ci == 0),
                                stop=(ci == nch - 1),
                            )
                else:
                    for ci in range(nch):
                        k = min(128, pix - ci * 128)
                        psumT = pt_pool.tile([128, 256], xdt, tag="pt")
                        for cb in range(2):
                            nc.tensor.transpose(
                                out=psumT[:k, cb * 128 : (cb + 1) * 128],
                                in_=csrc[:, cb * pix + ci * 128 : cb * pix + ci * 128 + k],
                                identity=ident[:],
                            )
                        for cb in range(2):
                            cropT = ct_pool.tile([128, 128], cdt, tag="ctp")
                            if (ci + cb) % 3 != 2:
                                nc.vector.tensor_copy(
                                    out=cropT[:k, :],
                                    in_=psumT[:k, cb * 128 : (cb + 1) * 128],
                                )
                            else:
                                nc.scalar.copy(
                                    out=cropT[:k, :],
                                    in_=psumT[:k, cb * 128 : (cb + 1) * 128],
                                )
                            nc.tensor.matmul(
                                out=pouts[cb][:],
                                lhsT=cropT[:k, :],
                                rhs=w_g[:k, (cio + ci) * NBINS : (cio + ci + 1) * NBINS],
                                start=(ci == 0),
                                stop=(ci == nch - 1),
                            )
                cio += nch
                j = t % GRP
                for cb in range(2):
                    nc.scalar.copy(
                        out=oacc[:, (j * 2 + cb) * NBINS : (j * 2 + cb + 1) * NBINS],
                        in_=pouts[cb][:],
                    )
                if t % GRP == GRP - 1 or t == n_slots - 1:
                    g0 = gi * GRP
                    gsz = t - g0 + 1
                    c0 = g0 * 2 * NBINS
                    nc.sync.dma_start(
                        out=out.ap()[:, c0 : c0 + gsz * 2 * NBINS],
                        in_=oacc[:, : gsz * 2 * NBINS],
                    )

    nc.finalize()
    return nc


# ---------------------------------------------------------------------------
# Entry point
# ---------------------------------------------------------------------------

def _run(inputs, trace=False, use_bf16=True):
    import ml_dtypes
    from concourse.bass_utils import run_bass_kernel_spmd

    feats_np = [
        np.ascontiguousarray(inputs[f"feat{l}"], dtype=np.float32) for l in range(4)
    ]
    boxes = np.asarray(inputs["boxes"], dtype=np.float32)
    batch_ids = np.asarray(inputs["batch_ids"])

    slots, tabs, wtabs, assign = _plan(boxes, batch_ids)
    nc = _build_program(slots, wtabs.shape[2], use_bf16=use_bf16)

    wdt = ml_dtypes.bfloat16 if use_bf16 else np.float32
    in_maps = []
    for k in range(NCORES):
        m = {f"feat{l}": feats_np[l] for l in range(4)}
        m["wtab"] = np.ascontiguousarray(wtabs[k].astype(wdt))
        m["tab"] = np.ascontiguousarray(tabs[k])
        in_maps.append(m)

    res = run_bass_kernel_spmd(nc, in_maps, core_ids=list(range(NCORES)), trace=trace)

    N = boxes.shape[0]
    n_slots = len(slots)
    result = np.zeros((N, C, P, P), dtype=np.float32)
    for k in range(NCORES):
        arr = res.results[k]["out"].reshape(128, n_slots, 2, NBINS)
        arr = arr.transpose(1, 2, 0, 3).reshape(n_slots, C, P, P)
        for t in range(n_slots):
            r = assign[k, t]
            if r >= 0:
                result[r] = arr[t]
    return result, res


def kernel(**inputs):
    result, _ = _run(inputs, trace=False)
    return result


# revision 26
# speedup vs baseline: 3.4984x; 3.4984x over previous
"""FPN RoIAlign pooler (maskrcnn-benchmark semantics) on 8 Trainium2 cores.

Strategy: data-parallel over RoIs. The host computes (from boxes/batch_ids
metadata only) each RoI's FPN level, feature-window origin/extent and a dense
interpolation matrix W[pix, 49] that folds bilinear weights + 2x2 sample
averaging. RoIs are packed into groups of 8 (one per core) with similar
window shapes, so one SPMD program (static shapes per slot = max over the
group) serves all cores; per-core variation flows through data tables
(window offsets, W values).

Device per slot: dynamic-offset DMA of the [2x128, KH, KW] feature window
into SBUF (channels on partitions), cast to bf16, TensorE transpose of
128-pixel chunks to pixel-major, then matmul cropT.T @ W accumulated into
PSUM [128, 49] per channel-block, copied into an output accumulator and
DMA'd out in groups.
"""

import random

import numpy as np

P = 7
SR = 2
S = P * SR
SCALES = (0.25, 0.125, 0.0625, 0.03125)
LSIZE = (200, 100, 50, 25)
C = 256
B = 2
NCORES = 8
NBINS = P * P
GRP = 8  # slots per W-prefetch / output-DMA group


def _map_levels(boxes):
    area = (boxes[:, 2] - boxes[:, 0]) * (boxes[:, 3] - boxes[:, 1])
    s = np.sqrt(np.maximum(area, 0.0))
    lvl = np.floor(4.0 + np.log2(s / 224.0 + 1e-6))
    return (np.clip(lvl, 2, 5) - 2).astype(np.int32)


def _plan(boxes, batch_ids):
    N = boxes.shape[0]
    boxes = boxes.astype(np.float64)
    lvl = _map_levels(boxes)
    bidx = batch_ids.astype(np.int64)

    sc = np.asarray(SCALES)[lvl]
    Ls = np.asarray(LSIZE)[lvl].astype(np.int64)  # H == W per level
    x1 = boxes[:, 0] * sc
    y1 = boxes[:, 1] * sc
    x2 = boxes[:, 2] * sc
    y2 = boxes[:, 3] * sc
    rw = np.maximum(x2 - x1, 1.0)
    rh = np.maximum(y2 - y1, 1.0)
    grid = (np.arange(P)[:, None] + (np.arange(SR)[None, :] + 0.5) / SR).reshape(-1)

    xs = x1[:, None] + grid[None, :] * (rw / P)[:, None]  # [N, S]
    ys = y1[:, None] + grid[None, :] * (rh / P)[:, None]
    Lf = Ls[:, None].astype(np.float64)
    vx = (xs > -1.0) & (xs < Lf)
    vy = (ys > -1.0) & (ys < Lf)
    xc = np.clip(xs, 0.0, Lf - 1.0)
    yc = np.clip(ys, 0.0, Lf - 1.0)
    xlo = np.minimum(np.floor(xc), Lf - 2.0).astype(np.int64)
    ylo = np.minimum(np.floor(yc), Lf - 2.0).astype(np.int64)
    fx = xc - xlo
    fy = yc - ylo

    # Tight window extents over valid samples (neighbors lo, lo+1).
    big = np.int64(1 << 40)
    x0 = np.where(vx, xlo, big).min(axis=1)
    xm = np.where(vx, xlo, -big).max(axis=1) + 1
    y0 = np.where(vy, ylo, big).min(axis=1)
    ym = np.where(vy, ylo, -big).max(axis=1) + 1
    dead = ~(vx.any(axis=1) & vy.any(axis=1))
    x0 = np.where(dead, 0, x0)
    xm = np.where(dead, 1, xm)
    y0 = np.where(dead, 0, y0)
    ym = np.where(dead, 1, ym)
    KW = (xm - x0 + 1).astype(np.int64)
    KH = (ym - y0 + 1).astype(np.int64)

    # --- pack RoIs into groups of NCORES with similar window shapes ---
    # DMA cost of a slot ~ KHmax*(KWmax + 24): one descriptor per (c, row),
    # each costing ~3 cycles + KW*4/32 beats.
    def gcost(ent):
        kh = max(KH[i] for i in ent)
        kw = max(KW[i] for i in ent)
        return kh * (kw + 24)

    groups = []
    for l in range(4):
        rem = [i for i in range(N) if lvl[i] == l]
        if not rem:
            continue
        rem.sort(key=lambda i: -(KH[i] * KW[i]))
        while rem:
            ent = [rem.pop(0)]
            while len(ent) < NCORES and rem:
                best = min(rem, key=lambda i: gcost(ent + [i]))
                rem.remove(best)
                ent.append(best)
            groups.append(ent)

    rng = random.Random(0)
    cost_cache = [gcost(e) for e in groups]
    ng = len(groups)
    for _ in range(200000):
        a = rng.randrange(ng)
        b = rng.randrange(ng)
        if a == b or lvl[groups[a][0]] != lvl[groups[b][0]]:
            continue
        ia = rng.randrange(len(groups[a]))
        ib = rng.randrange(len(groups[b]))
        ga = groups[a][:]
        gb = groups[b][:]
        ga[ia], gb[ib] = groups[b][ib], groups[a][ia]
        nc_ = gcost(ga) + gcost(gb)
        if nc_ < cost_cache[a] + cost_cache[b]:
            groups[a], groups[b] = ga, gb
            cost_cache[a], cost_cache[b] = gcost(ga), gcost(gb)

    # order: within each level, big slots first (stable pipeline warmup)
    groups.sort(key=lambda e: (lvl[e[0]], -gcost(e)))
    n_slots = len(groups)

    slot_lv = [int(lvl[e[0]]) for e in groups]
    slot_kh = [int(max(KH[i] for i in e)) for e in groups]
    slot_kw = [int(max(KW[i] for i in e)) for e in groups]

    pix = np.array([kh * kw for kh, kw in zip(slot_kh, slot_kw)], dtype=np.int64)
    nchunk = -(-pix // 128)
    wrows = nchunk * 128
    woff = np.concatenate([[0], np.cumsum(wrows)])

    tabs = np.zeros((NCORES, 1, n_slots), dtype=np.int32)
    totci = int(nchunk.sum())
    cioffs = np.concatenate([[0], np.cumsum(nchunk)]).astype(np.int64)
    wtabs = np.zeros((NCORES, 128, totci * NBINS), dtype=np.float32)
    assign = np.full((NCORES, n_slots), -1, dtype=np.int64)

    binq = (np.arange(S) // SR)[:, None] * P + (np.arange(S) // SR)[None, :]
    for t, ent in enumerate(groups):
        kh_s, kw_s = slot_kh[t], slot_kw[t]
        L = LSIZE[slot_lv[t]]
        HW = L * L
        for k in range(NCORES):
            if k >= len(ent):
                continue  # dummy: zero W, offset 0
            r = ent[k]
            assign[k, t] = r
            yo = min(int(y0[r]), L - kh_s)
            xo = min(int(x0[r]), L - kw_s)
            tabs[k, 0, t] = int(bidx[r]) * C * HW + yo * L + xo
            w = np.zeros((kh_s * kw_s, NBINS), dtype=np.float64)
            ry = (ylo[r] - yo).astype(np.int64)
            rx = (xlo[r] - xo).astype(np.int64)
            wy = np.stack([1.0 - fy[r], fy[r]])
            wx = np.stack([1.0 - fx[r], fx[r]])
            val = (vy[r][:, None] & vx[r][None, :]).astype(np.float64)
            for dy in range(2):
                for dx in range(2):
                    pixi = (ry + dy)[:, None] * kw_s + (rx + dx)[None, :]
                    wgt = wy[dy][:, None] * wx[dx][None, :] * 0.25 * val
                    np.add.at(w, (pixi.ravel(), binq.ravel()), wgt.ravel())
            wf = w.astype(np.float32)
            for ci in range(int(nchunk[t])):
                kk = min(128, kh_s * kw_s - ci * 128)
                col = (int(cioffs[t]) + ci) * NBINS
                wtabs[k, :kk, col : col + NBINS] = wf[ci * 128 : ci * 128 + kk]

    slots = [
        dict(
            lv=slot_lv[t],
            kh=slot_kh[t],
            kw=slot_kw[t],
            cio=int(cioffs[t]),
            nchunk=int(nchunk[t]),
        )
        for t in range(n_slots)
    ]
    return slots, tabs, wtabs, assign


# ---------------------------------------------------------------------------
# Device program
# ---------------------------------------------------------------------------

_MAXW = 1  # this walrus build allows 1 sem wait per TPB_CTRL (Drain)


def _patch_tile_drain():
    import concourse.tile as tile
    from bass_rust import ScopedClock

    if getattr(tile.TileContext, "_drain_patched", False):
        return

    def _drain_and_barrier(self, tick_clock, wait_clock):
        nc = self.nc
        drain_inst = nc.sync.drain()
        wait_clock.add_sem_waits(
            drain_inst.ins, ScopedClock({None: tick_clock.global_clock})
        )
        si = drain_inst.ins.sync_info
        ow = list(si.on_wait) if si is not None and si.on_wait else []
        if len(ow) > _MAXW:
            si.on_wait = ow[:_MAXW]
            for i in range(_MAXW, len(ow), _MAXW):
                d2 = nc.sync.drain()
                si2 = d2.ins.sync_info
                chunk = ow[i : i + _MAXW]
                if si2 is None:
                    d2.ins.sync_info = type(si)(on_wait=chunk, on_update=[])
                else:
                    si2.on_wait = chunk
        nc.all_engine_barrier()
        assert self.sems is not None
        popped = nc._tile_sem_poison_stack.pop()
        assert popped is self._sem_poison
        nc.clear_and_free_semaphores(list(self.sems.allocated().values()))
        nc.all_engine_barrier()

    tile.TileContext._drain_and_barrier = _drain_and_barrier
    tile.TileContext._drain_patched = True


def _load_idx(eng, ap, lo, hi, name):
    tmp = eng.alloc_register(f"idx_{name}")
    eng.reg_load(tmp, ap)
    return eng.snap(tmp, donate=True, min_val=lo, max_val=hi)


def _build_program(slots, wtab_cols, use_bf16=True, xpose_bf16=False, swdge=True, xbar=False):
    import concourse.bass as bass
    import concourse.tile as tile
    from concourse import bacc, mybir
    from concourse.masks import make_identity

    _patch_tile_drain()

    n_slots = len(slots)
    f32 = mybir.dt.float32
    cdt = mybir.dt.bfloat16 if use_bf16 else f32
    nc = bacc.Bacc("TRN2", target_bir_lowering=False, debug=False, num_devices=NCORES)

    feats = [
        nc.dram_tensor(f"feat{l}", [B, C, LSIZE[l], LSIZE[l]], f32, kind="ExternalInput")
        for l in range(4)
    ]
    wtab = nc.dram_tensor("wtab", [128, wtab_cols], cdt, kind="ExternalInput")
    tab = nc.dram_tensor("tab", [1, n_slots], mybir.dt.int32, kind="ExternalInput")
    out = nc.dram_tensor("out", [128, n_slots * 2 * NBINS], f32, kind="ExternalOutput")

    max_pix = max(s["kh"] * s["kw"] for s in slots)
    max_c16 = max(s["kh"] * s["kw"] + s["nchunk"] * 128 for s in slots)
    g_bounds = [(g0, min(g0 + GRP, n_slots)) for g0 in range(0, n_slots, GRP)]
    max_gci = max(sum(slots[t]["nchunk"] for t in range(g0, g1)) for g0, g1 in g_bounds)

    with tile.TileContext(nc) as tc:
        with (
            tc.tile_pool(name="const", bufs=1) as const_pool,
            tc.tile_pool(name="crop", bufs=6) as crop_pool,
            tc.tile_pool(name="crop16", bufs=4) as crop16_pool,
            tc.tile_pool(name="wsb", bufs=3) as w_pool,
            tc.tile_pool(name="ctp", bufs=12) as ct_pool,
            tc.tile_pool(name="oacc", bufs=3) as oacc_pool,
            tc.tile_pool(name="pt", bufs=4, space="PSUM") as pt_pool,
            tc.tile_pool(name="pout", bufs=4, space="PSUM") as pout_pool,
        ):
            xdt = cdt if xpose_bf16 else f32
            ident = const_pool.tile([128, 128], xdt)
            make_identity(nc, ident[:])
            tab_sb = const_pool.tile([1, n_slots], mybir.dt.int32)
            nc.sync.dma_start(tab_sb[:], tab.ap())

            oacc = None
            w_g = None
            cio = 0
            for t, sl in enumerate(slots):
                lv, kh, kw, nch = sl["lv"], sl["kh"], sl["kw"], sl["nchunk"]
                pix = kh * kw
                L = LSIZE[lv]
                HW = L * L
                gi = t // GRP
                if t % GRP == 0:
                    g0, g1 = g_bounds[gi]
                    gsz = g1 - g0
                    oacc = oacc_pool.tile([128, gsz * 2 * NBINS], f32, tag="oacc")
                    gci = sum(slots[u]["nchunk"] for u in range(g0, g1))
                    w_g = w_pool.tile([128, max_gci * NBINS], cdt, tag="wsb")
                    col0 = slots[g0]["cio"] * NBINS
                    nc.scalar.dma_start(
                        out=w_g[:, : gci * NBINS],
                        in_=wtab.ap()[:, col0 : col0 + gci * NBINS],
                    )
                    cio = 0

                max_off = (B - 1) * C * HW + (L - kh) * L + (L - kw)
                voff = _load_idx(nc.sync, tab_sb[0:1, t : t + 1], 0, max_off, f"o{t}")
                both_gp = swdge and t % 3 == 2
                if swdge:
                    voff2 = _load_idx(
                        nc.gpsimd, tab_sb[0:1, t : t + 1], 0, max_off, f"p{t}"
                    )
                else:
                    voff2 = voff
                crop = crop_pool.tile([128, 2 * max_pix], f32, tag="crop")
                for cb in range(2):
                    use_gp = swdge and (cb == 1 or both_gp)
                    src = bass.AP(
                        feats[lv],
                        (voff2 if use_gp else voff) + cb * 128 * HW,
                        [[HW, 128], [L, kh], [1, kw]],
                    )
                    eng = nc.gpsimd if use_gp else nc.sync
                    eng.dma_start(out=crop[:, cb * pix : (cb + 1) * pix], in_=src)

                if use_bf16 and (xpose_bf16 or xbar):
                    crop16 = crop16_pool.tile([128, max_c16], cdt, tag="crop16")
                    nc.vector.tensor_copy(
                        out=crop16[:, : 2 * pix], in_=crop[:, : 2 * pix]
                    )
                    tail = nch * 128 - pix
                    if xbar and tail > 0:
                        nc.vector.memset(crop16[:, 2 * pix : 2 * pix + tail], 0.0)
                    csrc = crop16
                else:
                    csrc = crop

                pouts = []
                for cb in range(2):
                    pout_cb = pout_pool.tile([128, NBINS], f32, tag="pout")
                    pouts.append(pout_cb)
                if xbar:
                    cropTs = []
                    for ci in range(nch):
                        for cb in range(2):
                            cropT = ct_pool.tile([128, 128], cdt, tag="ctp")
                            nc.scalar.dma_start(
                                out=cropT[:, :],
                                in_=csrc[
                                    :, cb * pix + ci * 128 : cb * pix + ci * 128 + 128
                                ],
                                transpose=True,
                            )
                            cropTs.append(cropT)
                    for ci in range(nch):
                        k = min(128, pix - ci * 128)
                        for cb in range(2):
                            nc.tensor.matmul(
                                out=pouts[cb][:],
                                lhsT=cropTs[ci * 2 + cb][:k, :],
                                rhs=w_g[
                                    :k, (cio + ci) * NBINS : (cio + ci + 1) * NBINS
                                ],
                                start=(ci == 0),
                                stop=(ci == nch - 1),
                            )
                else:
                    for ci in range(nch):
                        k = min(128, pix - ci * 128)
                        psumT = pt_pool.tile([128, 256], xdt, tag="pt")
                        for cb in range(2):
                            nc.tensor.transpose(
                                out=psumT[:k, cb * 128 : (cb + 1) * 128],
                                in_=csrc[:, cb * pix + ci * 128 : cb * pix + ci * 128 + k],
                                identity=ident[:],
                            )
                        cropT = ct_pool.tile([128, 256], cdt, tag="ctp")
                        if ci % 3 != 2:
                            nc.vector.tensor_copy(out=cropT[:k, :], in_=psumT[:k, :])
                        else:
                            nc.scalar.copy(out=cropT[:k, :], in_=psumT[:k, :])
                        for cb in range(2):
                            nc.tensor.matmul(
                                out=pouts[cb][:],
                                lhsT=cropT[:k, cb * 128 : (cb + 1) * 128],
                                rhs=w_g[:k, (cio + ci) * NBINS : (cio + ci + 1) * NBINS],
                                start=(ci == 0),
                                stop=(ci == nch - 1),
                            )
                cio += nch
                j = t % GRP
                for cb in range(2):
                    nc.scalar.copy(
                        out=oacc[:, (j * 2 + cb) * NBINS : (j * 2 + cb + 1) * NBINS],
                        in_=pouts[cb][:],
                    )
                if t % GRP == GRP - 1 or t == n_slots - 1:
                    g0 = gi * GRP
                    gsz = t - g0 + 1
                    c0 = g0 * 2 * NBINS
                    nc.sync.dma_start(
                        out=out.ap()[:, c0 : c0 + gsz * 2 * NBINS],
                        in_=oacc[:, : gsz * 2 * NBINS],
                    )

    nc.finalize()
    return nc


# ---------------------------------------------------------------------------
# Entry point
# ---------------------------------------------------------------------------

def _run(inputs, trace=False, use_bf16=True):
    import ml_dtypes
    from concourse.bass_utils import run_bass_kernel_spmd

    feats_np = [
        np.ascontiguousarray(inputs[f"feat{l}"], dtype=np.float32) for l in range(4)
    ]
    boxes = np.asarray(inputs["boxes"], dtype=np.float32)
    batch_ids = np.asarray(inputs["batch_ids"])

    slots, tabs, wtabs, assign = _plan(boxes, batch_ids)
    nc = _build_program(slots, wtabs.shape[2], use_bf16=use_bf16)

    wdt = ml_dtypes.bfloat16 if use_bf16 else np.float32
    in_maps = []
    for k in range(NCORES):
        m = {f"feat{l}": feats_np[l] for l in range(4)}
        m["wtab"] = np.ascontiguousarray(wtabs[k].astype(wdt))
        m["tab"] = np.ascontiguousarray(tabs[k])
        in_maps.append(m)

    res = run_bass_kernel_spmd(nc, in_maps, core_ids=list(range(NCORES)), trace=trace)

    N = boxes.shape[0]
    n_slots = len(slots)
    result = np.zeros((N, C, P, P), dtype=np.float32)
    for k in range(NCORES):
        arr = res.results[k]["out"].reshape(128, n_slots, 2, NBINS)
        arr = arr.transpose(1, 2, 0, 3).reshape(n_slots, C, P, P)
        for t in range(n_slots):
            r = assign[k, t]
            if r >= 0:
                result[r] = arr[t]
    return result, res


def kernel(**inputs):
    result, _ = _run(inputs, trace=False)
    return result
